# revision 29
# baseline (speedup 1.0000x reference)
"""DigitCapsules dynamic-routing kernel for 8 TRN2 NeuronCores.

Strategy: shard the input-capsule axis I=4096 across 8 cores (512 each).
Iteration 1 has uniform coupling, so its s_1 = (1/C)*sum_i u_hat is one
dense (i,d)-contraction: 64 PSUM-accumulated matmuls instead of a full
u_hat materialization.  Rounds 2-3 re-form u_hat per 4-pair "quad"
(block-diagonal K=32 matmuls, 4-way PE row strips), evacuate it to SBUF
bf16 on the scalar engine, and run the routing math with quad-batched
vector ops: fused-free D-reduction as a tree of 2x bf16 adds (D-major
free layout keeps every operand stride-1), softmax small ops, then a
c-weighted y that feeds selector matmuls accumulating s_j in PSUM.
b_ij is never stored: with b_0 = 0, b_r = u_hat . (v_1+...+v_{r-1}),
so only a running vsum [64,512] is kept.  Per round the tiny s partial
is AllReduced across the 8 cores.

B=64, I=4096, C=32, D=16, d=16, 3 routing iterations.
"""

import numpy as np

import concourse.bass as bass
import concourse.mybir as mybir
from concourse import library_config, tile
from concourse.bass_utils import run_bass_kernel_spmd

B = 64
I_FULL = 4096
C = 32
D = 16
DSMALL = 16
CD = C * D  # 512
NCORES = 8
I_LOC = I_FULL // NCORES  # 512
NPAIR = I_LOC // 2  # 256
NGRP = NPAIR // 4  # 64 groups of 4 pairs (one pair per 32-row strip)
NQ = NGRP  # quad q == weight group g
NCHUNK = 4  # input DMA chunks
GPC = NGRP // NCHUNK  # groups per chunk
EPS = 1e-9

F32 = mybir.dt.float32
BF16 = mybir.dt.bfloat16

AX = mybir.AxisListType.X
MUL = mybir.AluOpType.mult
ADD = mybir.AluOpType.add
COPY = mybir.ActivationFunctionType.Copy
EXP = mybir.ActivationFunctionType.Exp
SQRT = mybir.ActivationFunctionType.Sqrt


def _split_waits(nc, max_waits=1):
    """walrus in this toolchain rejects instructions carrying more than
    ~2 semaphore waits; move extras onto preceding same-engine NOPs."""
    for bb_wrap in nc.bb_map.values():
        bb = bb_wrap.bb
        newlist = []
        changed = False
        for inst in bb.instructions:
            si = inst.sync_info
            waits = list(si.on_wait) if si and si.on_wait else []
            if len(waits) > max_waits:
                extra, keep = waits[:-max_waits], waits[-max_waits:]
                k = 0
                while extra:
                    chunk, extra = extra[:max_waits], extra[max_waits:]
                    nop = mybir.InstNoOp(
                        name=f"{inst.name}-waitsplit{k}",
                        engine=inst.engine,
                        sync_info=mybir.SyncInfo(on_wait=chunk, on_update=[]),
                    )
                    nc.register_instruction(nop, overwrite=True)
                    newlist.append(nop)
                    k += 1
                inst.sync_info = mybir.SyncInfo(
                    on_wait=keep,
                    on_update=list(si.on_update) if si.on_update else [],
                )
                changed = True
            newlist.append(inst)
        if changed:
            bb.instructions = newlist


def build_bass():
    nc = bass.Bass(
        "TRN2", target_bir_lowering=False, debug=False, num_devices=NCORES
    )
    xd_ext = nc.dram_tensor("xd", [128, NGRP * 128], BF16, kind="ExternalInput").ap()
    wt_ext = nc.dram_tensor("wt", [128, NGRP * CD], BF16, kind="ExternalInput").ap()
    xs_ext = nc.dram_tensor("xs", [128, NGRP * B], BF16, kind="ExternalInput").ap()
    sel_ext = nc.dram_tensor("sel", [128, B], BF16, kind="ExternalInput").ap()
    out_ext = nc.dram_tensor("out", [B, CD], F32, kind="ExternalOutput").ap()

    with tile.TileContext(nc) as tc:
        with (
            tc.tile_pool(name="persist", bufs=1) as pp,
            tc.tile_pool(name="usb", bufs=12) as up,
            tc.tile_pool(name="tmp", bufs=2) as tp,
            tc.tile_pool(name="yp", bufs=4) as yp,
            tc.tile_pool(name="small", bufs=4) as sp,
            tc.tile_pool(name="bnd", bufs=1) as bp,
            tc.tile_pool(name="uh", bufs=3, space="PSUM") as uhp,
            tc.tile_pool(name="sacc", bufs=1, space="PSUM") as saccp,
            tc.tile_pool(name="dram", bufs=2, space="DRAM") as dp,
        ):
            # chunked inputs: one tile per DMA chunk so round-1 matmuls can
            # start before the whole weight tensor has landed
            xd_t, wt_t, xs_t = [], [], []
            for k in range(NCHUNK):
                xdk = pp.tile([128, GPC * 128], BF16, tag=f"xd{k}")
                wtk = pp.tile([128, GPC * CD], BF16, tag=f"wt{k}")
                xsk = pp.tile([128, GPC * B], BF16, tag=f"xs{k}")
                nc.sync.dma_start(wtk[:], wt_ext[:, k * GPC * CD : (k + 1) * GPC * CD])
                nc.sync.dma_start(xdk[:], xd_ext[:, k * GPC * 128 : (k + 1) * GPC * 128])
                nc.sync.dma_start(xsk[:], xs_ext[:, k * GPC * B : (k + 1) * GPC * B])
                xd_t.append(xdk)
                wt_t.append(wtk)
                xs_t.append(xsk)
            sel = pp.tile([128, B], BF16, tag="sel")
            nc.sync.dma_start(sel[:], sel_ext)
            vsum = pp.tile([B, CD], F32, tag="vsum")

            def wt_g(g):
                return wt_t[g // GPC][:, (g % GPC) * CD : (g % GPC + 1) * CD]

            def xd_g(g):
                return xd_t[g // GPC][:, (g % GPC) * 128 : (g % GPC + 1) * 128]

            def xs_g(g):
                return xs_t[g // GPC][:, (g % GPC) * B : (g % GPC + 1) * B]

            # ---- round 1: s1 = (1/C) sum_i u_hat  (dense (i,d) contraction)
            # split into two PSUM halves so the first half's all-reduce
            # overlaps the second half's matmuls
            s_psA = saccp.tile([B, CD], F32, tag="spsA")
            s_psB = saccp.tile([B, CD], F32, tag="spsB")
            for g in range(NGRP):
                s_ps = s_psA if g < NGRP // 2 else s_psB
                nc.tensor.matmul(
                    s_ps[:],
                    lhsT=xs_g(g),
                    rhs=wt_g(g),
                    start=(g % (NGRP // 2) == 0),
                    stop=(g % (NGRP // 2) == NGRP // 2 - 1),
                    skip_group_check=True,
                )

            def half_reduce(r, s_ps, half):
                """evacuate one s half to bf16 and launch its all-reduce.
                High priority: the trigger DMAs must jump the Pool queue
                (otherwise they trail every fold op of the round)."""
                with tc.high_priority():
                    s_sb = bp.tile([B, CD], BF16, tag=f"s_sb{half}")
                    # fold uniform c=1/C of round 1 into the evacuation
                    nc.scalar.activation(
                        s_sb[:], s_ps[:], COPY, scale=(1.0 / C if r == 1 else 1.0)
                    )
                    ccin = dp.tile([B, CD], BF16, tag=f"ccin{half}")
                    ccout = dp.tile([B, CD], BF16, tag=f"ccout{half}")
                    nc.gpsimd.dma_start(ccin[:], s_sb[:])
                    nc.gpsimd.collective_compute(
                        "AllReduce",
                        ADD,
                        replica_groups=[list(range(NCORES))],
                        ins=[ccin[:].opt()],
                        outs=[ccout[:].opt()],
                    )
                    sr = bp.tile([B, CD], BF16, tag=f"sr{half}")
                    nc.gpsimd.dma_start(sr[:], ccout[:])
                return sr

            def boundary(r, srA, srB):
                """combine the two reduced halves, squash; update vsum/vbc
                (rounds 1,2) or DMA the final output (round 3)."""
                s2 = bp.tile([B, CD], F32, tag="s2")
                nc.vector.tensor_tensor(out=s2[:], in0=srA[:], in1=srB[:], op=ADD)

                # squash: v = s/(1+n2)/sqrt(n2+eps); D-major free layout
                sq = bp.tile([B, CD], F32, tag="sq")
                nc.vector.tensor_tensor(out=sq[:], in0=s2[:], in1=s2[:], op=MUL)
                n2 = sp.tile([B, C], F32, tag="n2")
                nc.vector.tensor_reduce(
                    out=n2[:],
                    in_=sq[:].rearrange("b (d c) -> b c d", c=C),
                    axis=AX,
                    op=ADD,
                )
                n2e = sp.tile([B, C], F32, tag="n2e")
                nc.vector.tensor_scalar_add(n2e[:], n2[:], EPS)
                rt = sp.tile([B, C], F32, tag="rt")
                nc.scalar.activation(rt[:], n2e[:], SQRT)
                on2 = sp.tile([B, C], F32, tag="on2")
                nc.vector.tensor_scalar_add(on2[:], n2[:], 1.0)
                den = sp.tile([B, C], F32, tag="den")
                nc.vector.tensor_tensor(out=den[:], in0=rt[:], in1=on2[:], op=MUL)
                scl = sp.tile([B, C], F32, tag="scl")
                nc.vector.reciprocal(scl[:], den[:])

                if r == 3:
                    # v3 written c-major for the external output
                    v_out = bp.tile([B, CD], F32, tag="v_out")
                    nc.vector.tensor_tensor(
                        out=v_out[:].rearrange("b (c d) -> b c d", c=C),
                        in0=s2[:].rearrange("b (d c) -> b c d", c=C),
                        in1=scl[:].unsqueeze(2).broadcast_to([B, C, D]),
                        op=MUL,
                    )
                    nc.sync.dma_start(out_ext, v_out[:])
                    return None

                v_cur = bp.tile([B, CD], F32, tag="v_cur")
                nc.vector.tensor_tensor(
                    out=v_cur[:].rearrange("b (d c) -> b d c", c=C),
                    in0=s2[:].rearrange("b (d c) -> b d c", c=C),
                    in1=scl[:].unsqueeze(1).broadcast_to([B, D, C]),
                    op=MUL,
                )
                if r == 1:
                    nc.vector.tensor_scalar_mul(vsum[:], v_cur[:], 1.0)
                else:
                    nc.vector.tensor_tensor(
                        out=vsum[:], in0=vsum[:], in1=v_cur[:], op=ADD
                    )
                v_bf = bp.tile([B, CD], BF16, tag="v_bf")
                nc.scalar.activation(v_bf[:], vsum[:], COPY)
                vbc = bp.tile([128, CD], BF16, tag="vbc")
                nc.gpsimd.dma_start(vbc[0:B, :], v_bf[:])
                nc.gpsimd.dma_start(vbc[B : 2 * B, :], v_bf[:])
                return vbc

            # warm up the collective channel early: the first AllReduce
            # pays ~50us of setup; overlap it with round 1's matmuls
            # warm both real collective channels (same tags/shape/dtype as
            # the round reductions) so the first boundary pays no setup
            wzero = bp.tile([B, CD], BF16, tag="wzero")
            with tc.high_priority():
                nc.gpsimd.memset(wzero[:], 0.0)
                for half in ("A", "B"):
                    ccw_in = dp.tile([B, CD], BF16, tag=f"ccin{half}", name=f"ccwi{half}")
                    ccw_out = dp.tile([B, CD], BF16, tag=f"ccout{half}", name=f"ccwo{half}")
                    nc.gpsimd.dma_start(ccw_in[:], wzero[:])
                    nc.gpsimd.collective_compute(
                        "AllReduce",
                        ADD,
                        replica_groups=[list(range(NCORES))],
                        ins=[ccw_in[:].opt()],
                        outs=[ccw_out[:].opt()],
                    )
                    wres = bp.tile([B, CD], BF16, tag=f"wres{half}", name=f"wres{half}")
                    nc.gpsimd.dma_start(wres[:], ccw_out[:])

            # ---- rounds 2,3 (software-pipelined; u-matmuls+evacuation run
            # PRE quads ahead so they cover the round-boundary allreduce)
            PRE = 8
            VBC = [None]
            st_u = {}  # q -> u_sb
            st_a = {}  # q -> (u_sb, a)
            st_b = {}  # q -> (u_sb, cn)
            st_c = {}  # q -> y
            SPS = [None]

            def stage_u(q):
                uh0 = uhp.tile([128, 2 * CD], F32, tag="uh")
                uh1 = uhp.tile([128, 2 * CD], F32, tag="uh")
                for st in range(4):
                    dst = (uh0 if st < 2 else uh1)[
                        :, (st % 2) * CD : (st % 2 + 1) * CD
                    ]
                    nc.tensor.matmul(
                        dst,
                        lhsT=xd_g(q)[32 * st : 32 * st + 32, :],
                        rhs=wt_g(q)[32 * st : 32 * st + 32, :],
                        start=True,
                        stop=True,
                        tile_position=(32 * st, 0),
                    )
                u_sb = up.tile([128, 4 * CD], BF16, tag="usb")
                nc.scalar.activation(u_sb[:, 0 : 2 * CD], uh0[:], COPY)
                nc.scalar.activation(u_sb[:, 2 * CD : 4 * CD], uh1[:], COPY)
                st_u[q] = u_sb

            def stage_v(q):
                u_sb = st_u.pop(q)
                vbc = VBC[0]
                # agreement: tmp = u_hat * vsum, then sum over D
                # (D-major: every fold reads/writes stride-1 bf16 -> 2x)
                tmp = tp.tile([128, 4 * CD], BF16, tag="tmp")
                nc.vector.tensor_tensor(
                    out=tmp[:].rearrange("p (q n) -> p q n", n=CD),
                    in0=u_sb[:].rearrange("p (q n) -> p q n", n=CD),
                    in1=vbc[:].unsqueeze(1).broadcast_to([128, 4, CD]),
                    op=MUL,
                )
                t1 = tp.tile([128, 4 * 8 * C], BF16, tag="t1")
                v4 = tmp[:].rearrange("p (q d c) -> p q d c", d=D, c=C)
                o1 = t1[:].rearrange("p (q d c) -> p q d c", d=8, c=C)
                nc.vector.tensor_tensor(
                    out=o1, in0=v4[:, :, 0:8, :], in1=v4[:, :, 8:16, :], op=ADD
                )
                t2 = sp.tile([128, 4 * 4 * C], BF16, tag="t2")
                i2 = t1[:].rearrange("p (q d c) -> p q d c", d=8, c=C)
                o2 = t2[:].rearrange("p (q d c) -> p q d c", d=4, c=C)
                nc.vector.tensor_tensor(
                    out=o2, in0=i2[:, :, 0:4, :], in1=i2[:, :, 4:8, :], op=ADD
                )
                t3 = sp.tile([128, 4 * 2 * C], BF16, tag="t3")
                i3 = t2[:].rearrange("p (q d c) -> p q d c", d=4, c=C)
                o3 = t3[:].rearrange("p (q d c) -> p q d c", d=2, c=C)
                nc.gpsimd.tensor_tensor(
                    out=o3, in0=i3[:, :, 0:2, :], in1=i3[:, :, 2:4, :], op=ADD
                )
                a = sp.tile([128, 4 * C], F32, tag="a")
                i4 = t3[:].rearrange("p (q d c) -> p q d c", d=2, c=C)
                nc.gpsimd.tensor_tensor(
                    out=a[:].rearrange("p (q c) -> p q c", c=C).unsqueeze(2),
                    in0=i4[:, :, 0:1, :],
                    in1=i4[:, :, 1:2, :],
                    op=ADD,
                )
                st_a[q] = (u_sb, a)

            def stage_b(q):
                u_sb, a = st_a.pop(q)
                e = sp.tile([128, 4 * C], F32, tag="e")
                sm = sp.tile([128, 4], F32, tag="sm")
                cn = sp.tile([128, 4 * C], BF16, tag="cn")
                for k in range(4):
                    # exp with free-dim accumulate -> per-pair softmax denom
                    nc.scalar.activation(
                        e[:, k * C : (k + 1) * C],
                        a[:, k * C : (k + 1) * C],
                        EXP,
                        accum_out=sm[:, k : k + 1],
                    )
                rs = sp.tile([128, 4], F32, tag="rs")
                nc.vector.reciprocal(rs[:], sm[:])
                nc.vector.tensor_tensor(
                    out=cn[:].rearrange("p (q c) -> p q c", c=C),
                    in0=e[:].rearrange("p (q c) -> p q c", c=C),
                    in1=rs[:].unsqueeze(2).broadcast_to([128, 4, C]),
                    op=MUL,
                )
                st_b[q] = (u_sb, cn)

            def stage_y(q):
                u_sb, cn = st_b.pop(q)
                y = yp.tile([128, 4 * CD], BF16, tag="y")
                # cn broadcast along D stays stride-1 in the D-major layout
                nc.vector.tensor_tensor(
                    out=y[:].rearrange("p (q d c) -> p q d c", d=D, c=C),
                    in0=u_sb[:].rearrange("p (q d c) -> p q d c", d=D, c=C),
                    in1=cn[:]
                    .rearrange("p (q c) -> p q c", c=C)
                    .unsqueeze(2)
                    .broadcast_to([128, 4, D, C]),
                    op=MUL,
                )
                st_c[q] = y

            def stage_c(q):
                y = st_c.pop(q)
                sA, sB = SPS[0]
                half = NQ // 2
                s_ps = sA if q < half else sB
                for h in range(4):
                    nc.tensor.matmul(
                        s_ps[:],
                        lhsT=sel[:],
                        rhs=y[:, h * CD : (h + 1) * CD],
                        start=(q % half == 0 and h == 0),
                        stop=(q % half == half - 1 and h == 3),
                        skip_group_check=True,
                    )

            srA = half_reduce(1, s_psA, "A")
            for q in range(PRE):
                stage_u(q)
            srB = half_reduce(1, s_psB, "B")
            VBC[0] = boundary(1, srA, srB)
            HALF = NQ // 2
            for r in (2, 3):
                sA = saccp.tile([B, CD], F32, tag="spsA", name="spsA_r")
                sB = saccp.tile([B, CD], F32, tag="spsB", name="spsB_r")
                SPS[0] = (sA, sB)
                srA = None
                for qq in range(0, NQ + 8, 2):
                    for q in (qq, qq + 1):
                        if q + PRE < NQ:
                            stage_u(q + PRE)
                    for q in (qq, qq + 1):
                        if q < NQ:
                            stage_v(q)
                    for q in (qq - 2, qq - 1):
                        if 0 <= q < NQ:
                            stage_b(q)
                    for q in (qq - 4, qq - 3):
                        if 0 <= q < NQ:
                            stage_y(q)
                    for q in (qq - 6, qq - 5):
                        if 0 <= q < NQ:
                            stage_c(q)
                    if qq - 6 == HALF - 2:
                        # first-half s complete: launch its all-reduce now
                        srA = half_reduce(r, sA, "A")
                if r == 2:
                    for q in range(PRE):
                        stage_u(q)
                srB = half_reduce(r, sB, "B")
                VBC[0] = boundary(r, srA, srB)
    _split_waits(nc)
    return nc


def _prep_core_inputs(x_np, w_np, core):
    """x_np [B, I, d] f32; w_np [I, C, D, d] f32 -> per-core bf16 operands.

    Free-dim layout for u_hat tiles is D-major: n = D*C + c."""
    import ml_dtypes

    lo = core * I_LOC
    xk = x_np[:, lo : lo + I_LOC, :]  # [B, 512, 16]
    wk = w_np[lo : lo + I_LOC]  # [512, C, D, d]

    # W rows per i: [d, (D, c)] (D-major free)
    w_free = wk.transpose(0, 3, 2, 1).reshape(I_LOC, DSMALL, CD)  # [i, d, (D c)]
    # pair tiles [NPAIR, 32, CD]; rows 0:16 = i0 over d, 16:32 = i1
    wt = np.zeros((NPAIR, 32, CD), dtype=np.float32)
    wt[:, 0:DSMALL, :] = w_free[0::2]
    wt[:, DSMALL:32, :] = w_free[1::2]
    # strip-pack: pair p=4g+s -> partitions [32s,32s+32), free block g
    wsb = wt.reshape(NGRP, 4, 32, CD).transpose(1, 2, 0, 3).reshape(128, NGRP * CD)

    # x block-diag pair tiles: [NPAIR, 32, 128]
    xdg = np.zeros((NPAIR, 32, 128), dtype=np.float32)
    xT = xk.transpose(1, 2, 0)  # [i, d, B]
    xdg[:, 0:DSMALL, 0:B] = xT[0::2]
    xdg[:, DSMALL:32, B : 2 * B] = xT[1::2]
    xsb = xdg.reshape(NGRP, 4, 32, 128).transpose(1, 2, 0, 3).reshape(128, NGRP * 128)

    # dense x chunks for round 1: rows (j, d) = i 8g+j; cols (g, b)
    xs = xT.reshape(NGRP, 8 * DSMALL, B).transpose(1, 0, 2).reshape(128, NGRP * B)

    return {
        "xd": xsb.astype(ml_dtypes.bfloat16),
        "wt": wsb.astype(ml_dtypes.bfloat16),
        "xs": xs.astype(ml_dtypes.bfloat16),
    }


_NC_CACHE = {}


def kernel(x: np.ndarray, weights: np.ndarray) -> np.ndarray:
    import ml_dtypes

    x = np.asarray(x, dtype=np.float32)
    w = np.asarray(weights, dtype=np.float32)[0]  # [I, C, D, d]

    if "nc" not in _NC_CACHE:
        _NC_CACHE["nc"] = build_bass()
    nc = _NC_CACHE["nc"]

    selmask = np.zeros((128, B), dtype=np.float32)
    for p in range(128):
        selmask[p, p % B] = 1.0

    in_maps = []
    for core in range(NCORES):
        m = _prep_core_inputs(x, w, core)
        m["sel"] = selmask.astype(ml_dtypes.bfloat16)
        in_maps.append(m)

    res = run_bass_kernel_spmd(nc, in_maps, list(range(NCORES)))
    out = np.asarray(res.results[0]["out"], dtype=np.float32)  # [B, CD]
    return out.reshape(B, C, D)


# revision 30
# speedup vs baseline: 1.0612x; 1.0612x over previous
"""DigitCapsules dynamic-routing kernel for 8 TRN2 NeuronCores.

Strategy: shard the input-capsule axis I=4096 across 8 cores (512 each).
Iteration 1 has uniform coupling, so its s_1 = (1/C)*sum_i u_hat is one
dense (i,d)-contraction: 64 PSUM-accumulated matmuls instead of a full
u_hat materialization.  Rounds 2-3 re-form u_hat per 4-pair "quad"
(block-diagonal K=32 matmuls, 4-way PE row strips), evacuate it to SBUF
bf16 on the scalar engine, and run the routing math with quad-batched
vector ops: fused-free D-reduction as a tree of 2x bf16 adds (D-major
free layout keeps every operand stride-1), softmax small ops, then a
c-weighted y that feeds selector matmuls accumulating s_j in PSUM.
b_ij is never stored: with b_0 = 0, b_r = u_hat . (v_1+...+v_{r-1}),
so only a running vsum [64,512] is kept.  Per round the tiny s partial
is AllReduced across the 8 cores.

B=64, I=4096, C=32, D=16, d=16, 3 routing iterations.
"""

import numpy as np

import concourse.bass as bass
import concourse.mybir as mybir
from concourse import library_config, tile
from concourse.bass_utils import run_bass_kernel_spmd

B = 64
I_FULL = 4096
C = 32
D = 16
DSMALL = 16
CD = C * D  # 512
NCORES = 8
I_LOC = I_FULL // NCORES  # 512
NPAIR = I_LOC // 2  # 256
NGRP = NPAIR // 4  # 64 groups of 4 pairs (one pair per 32-row strip)
NQ = NGRP  # quad q == weight group g
NCHUNK = 4  # input DMA chunks
GPC = NGRP // NCHUNK  # groups per chunk
EPS = 1e-9

F32 = mybir.dt.float32
BF16 = mybir.dt.bfloat16

AX = mybir.AxisListType.X
MUL = mybir.AluOpType.mult
ADD = mybir.AluOpType.add
COPY = mybir.ActivationFunctionType.Copy
EXP = mybir.ActivationFunctionType.Exp
SQRT = mybir.ActivationFunctionType.Sqrt


def _split_waits(nc, max_waits=1):
    """walrus in this toolchain rejects instructions carrying more than
    ~2 semaphore waits; move extras onto preceding same-engine NOPs."""
    for bb_wrap in nc.bb_map.values():
        bb = bb_wrap.bb
        newlist = []
        changed = False
        for inst in bb.instructions:
            si = inst.sync_info
            waits = list(si.on_wait) if si and si.on_wait else []
            if len(waits) > max_waits:
                extra, keep = waits[:-max_waits], waits[-max_waits:]
                k = 0
                while extra:
                    chunk, extra = extra[:max_waits], extra[max_waits:]
                    nop = mybir.InstNoOp(
                        name=f"{inst.name}-waitsplit{k}",
                        engine=inst.engine,
                        sync_info=mybir.SyncInfo(on_wait=chunk, on_update=[]),
                    )
                    nc.register_instruction(nop, overwrite=True)
                    newlist.append(nop)
                    k += 1
                inst.sync_info = mybir.SyncInfo(
                    on_wait=keep,
                    on_update=list(si.on_update) if si.on_update else [],
                )
                changed = True
            newlist.append(inst)
        if changed:
            bb.instructions = newlist


def build_bass():
    nc = bass.Bass(
        "TRN2", target_bir_lowering=False, debug=False, num_devices=NCORES
    )
    xd_ext = nc.dram_tensor("xd", [128, NGRP * 128], BF16, kind="ExternalInput").ap()
    wt_ext = nc.dram_tensor("wt", [128, NGRP * CD], BF16, kind="ExternalInput").ap()
    xs_ext = nc.dram_tensor("xs", [128, NGRP * B], BF16, kind="ExternalInput").ap()
    sel_ext = nc.dram_tensor("sel", [128, B], BF16, kind="ExternalInput").ap()
    out_ext = nc.dram_tensor("out", [B, CD], F32, kind="ExternalOutput").ap()

    with tile.TileContext(nc) as tc:
        with (
            tc.tile_pool(name="persist", bufs=1) as pp,
            tc.tile_pool(name="usb", bufs=12) as up,
            tc.tile_pool(name="tmp", bufs=2) as tp,
            tc.tile_pool(name="yp", bufs=4) as yp,
            tc.tile_pool(name="small", bufs=4) as sp,
            tc.tile_pool(name="bnd", bufs=1) as bp,
            tc.tile_pool(name="uh", bufs=3, space="PSUM") as uhp,
            tc.tile_pool(name="sacc", bufs=1, space="PSUM") as saccp,
            tc.tile_pool(name="dram", bufs=2, space="DRAM") as dp,
        ):
            # chunked inputs: one tile per DMA chunk so round-1 matmuls can
            # start before the whole weight tensor has landed
            xd_t, wt_t, xs_t = [], [], []
            for k in range(NCHUNK):
                xdk = pp.tile([128, GPC * 128], BF16, tag=f"xd{k}")
                wtk = pp.tile([128, GPC * CD], BF16, tag=f"wt{k}")
                xsk = pp.tile([128, GPC * B], BF16, tag=f"xs{k}")
                nc.sync.dma_start(wtk[:], wt_ext[:, k * GPC * CD : (k + 1) * GPC * CD])
                nc.sync.dma_start(xdk[:], xd_ext[:, k * GPC * 128 : (k + 1) * GPC * 128])
                nc.sync.dma_start(xsk[:], xs_ext[:, k * GPC * B : (k + 1) * GPC * B])
                xd_t.append(xdk)
                wt_t.append(wtk)
                xs_t.append(xsk)
            sel = pp.tile([128, B], BF16, tag="sel")
            nc.sync.dma_start(sel[:], sel_ext)
            vsum = pp.tile([B, CD], F32, tag="vsum")

            def wt_g(g):
                return wt_t[g // GPC][:, (g % GPC) * CD : (g % GPC + 1) * CD]

            def xd_g(g):
                return xd_t[g // GPC][:, (g % GPC) * 128 : (g % GPC + 1) * 128]

            def xs_g(g):
                return xs_t[g // GPC][:, (g % GPC) * B : (g % GPC + 1) * B]

            # ---- round 1: s1 = (1/C) sum_i u_hat  (dense (i,d) contraction)
            # split into two PSUM halves so the first half's all-reduce
            # overlaps the second half's matmuls
            s_psA = saccp.tile([B, CD], F32, tag="spsA")
            s_psB = saccp.tile([B, CD], F32, tag="spsB")
            for g in range(NGRP):
                s_ps = s_psA if g < NGRP // 2 else s_psB
                nc.tensor.matmul(
                    s_ps[:],
                    lhsT=xs_g(g),
                    rhs=wt_g(g),
                    start=(g % (NGRP // 2) == 0),
                    stop=(g % (NGRP // 2) == NGRP // 2 - 1),
                    skip_group_check=True,
                )

            def half_reduce(r, s_ps, half):
                """evacuate one s half to bf16 and launch its all-reduce.
                High priority: the trigger DMAs must jump the Pool queue
                (otherwise they trail every fold op of the round)."""
                with tc.high_priority():
                    s_sb = bp.tile([B, CD], BF16, tag=f"s_sb{half}")
                    # fold uniform c=1/C of round 1 into the evacuation
                    nc.scalar.activation(
                        s_sb[:], s_ps[:], COPY, scale=(1.0 / C if r == 1 else 1.0)
                    )
                    ccin = dp.tile([B, CD], BF16, tag=f"ccin{half}")
                    ccout = dp.tile([B, CD], BF16, tag=f"ccout{half}")
                    nc.gpsimd.dma_start(ccin[:], s_sb[:])
                    nc.gpsimd.collective_compute(
                        "AllReduce",
                        ADD,
                        replica_groups=[list(range(NCORES))],
                        ins=[ccin[:].opt()],
                        outs=[ccout[:].opt()],
                    )
                    sr = bp.tile([B, CD], BF16, tag=f"sr{half}")
                    nc.gpsimd.dma_start(sr[:], ccout[:])
                return sr

            def boundary(r, srA, srB):
                """combine the two reduced halves, squash; update vsum/vbc
                (rounds 1,2) or DMA the final output (round 3)."""
                s2 = bp.tile([B, CD], F32, tag="s2")
                nc.vector.tensor_tensor(out=s2[:], in0=srA[:], in1=srB[:], op=ADD)

                # squash: v = s/(1+n2)/sqrt(n2+eps); D-major free layout
                sq = bp.tile([B, CD], F32, tag="sq")
                nc.vector.tensor_tensor(out=sq[:], in0=s2[:], in1=s2[:], op=MUL)
                n2 = sp.tile([B, C], F32, tag="n2")
                nc.vector.tensor_reduce(
                    out=n2[:],
                    in_=sq[:].rearrange("b (d c) -> b c d", c=C),
                    axis=AX,
                    op=ADD,
                )
                n2e = sp.tile([B, C], F32, tag="n2e")
                nc.vector.tensor_scalar_add(n2e[:], n2[:], EPS)
                rt = sp.tile([B, C], F32, tag="rt")
                nc.scalar.activation(rt[:], n2e[:], SQRT)
                on2 = sp.tile([B, C], F32, tag="on2")
                nc.vector.tensor_scalar_add(on2[:], n2[:], 1.0)
                den = sp.tile([B, C], F32, tag="den")
                nc.vector.tensor_tensor(out=den[:], in0=rt[:], in1=on2[:], op=MUL)
                scl = sp.tile([B, C], F32, tag="scl")
                nc.vector.reciprocal(scl[:], den[:])

                if r == 3:
                    # v3 written c-major for the external output
                    v_out = bp.tile([B, CD], F32, tag="v_out")
                    nc.vector.tensor_tensor(
                        out=v_out[:].rearrange("b (c d) -> b c d", c=C),
                        in0=s2[:].rearrange("b (d c) -> b c d", c=C),
                        in1=scl[:].unsqueeze(2).broadcast_to([B, C, D]),
                        op=MUL,
                    )
                    nc.sync.dma_start(out_ext, v_out[:])
                    return None

                v_cur = bp.tile([B, CD], F32, tag="v_cur")
                nc.vector.tensor_tensor(
                    out=v_cur[:].rearrange("b (d c) -> b d c", c=C),
                    in0=s2[:].rearrange("b (d c) -> b d c", c=C),
                    in1=scl[:].unsqueeze(1).broadcast_to([B, D, C]),
                    op=MUL,
                )
                if r == 1:
                    nc.vector.tensor_scalar_mul(vsum[:], v_cur[:], 1.0)
                else:
                    nc.vector.tensor_tensor(
                        out=vsum[:], in0=vsum[:], in1=v_cur[:], op=ADD
                    )
                v_bf = bp.tile([B, CD], BF16, tag="v_bf")
                nc.scalar.activation(v_bf[:], vsum[:], COPY)
                vbc = bp.tile([128, CD], BF16, tag="vbc")
                nc.gpsimd.dma_start(vbc[0:B, :], v_bf[:])
                nc.gpsimd.dma_start(vbc[B : 2 * B, :], v_bf[:])
                return vbc

            # warm up the collective channel early: the first AllReduce
            # pays ~50us of setup; overlap it with round 1's matmuls
            # warm up the collective channel early: the first AllReduce
            # pays tens of us of setup; overlap it with round 1's matmuls
            ccw_in = dp.tile([B, 16], F32, tag="ccwin")
            ccw_out = dp.tile([B, 16], F32, tag="ccwout")
            wzero = bp.tile([B, 16], F32, tag="wzero")
            wres = bp.tile([B, 16], F32, tag="wres")
            with tc.high_priority():
                nc.gpsimd.memset(wzero[:], 0.0)
                nc.gpsimd.dma_start(ccw_in[:], wzero[:])
                nc.gpsimd.collective_compute(
                    "AllReduce",
                    ADD,
                    replica_groups=[list(range(NCORES))],
                    ins=[ccw_in[:].opt()],
                    outs=[ccw_out[:].opt()],
                )
                nc.gpsimd.dma_start(wres[:], ccw_out[:])

            # ---- rounds 2,3 (software-pipelined; u-matmuls+evacuation run
            # PRE quads ahead so they cover the round-boundary allreduce)
            PRE = 8
            VBC = [None]
            st_u = {}  # q -> u_sb
            st_a = {}  # q -> (u_sb, a)
            st_b = {}  # q -> (u_sb, cn)
            st_c = {}  # q -> y
            SPS = [None]

            def stage_u(q):
                uh0 = uhp.tile([128, 2 * CD], F32, tag="uh")
                uh1 = uhp.tile([128, 2 * CD], F32, tag="uh")
                for st in range(4):
                    dst = (uh0 if st < 2 else uh1)[
                        :, (st % 2) * CD : (st % 2 + 1) * CD
                    ]
                    nc.tensor.matmul(
                        dst,
                        lhsT=xd_g(q)[32 * st : 32 * st + 32, :],
                        rhs=wt_g(q)[32 * st : 32 * st + 32, :],
                        start=True,
                        stop=True,
                        tile_position=(32 * st, 0),
                    )
                u_sb = up.tile([128, 4 * CD], BF16, tag="usb")
                nc.scalar.activation(u_sb[:, 0 : 2 * CD], uh0[:], COPY)
                nc.scalar.activation(u_sb[:, 2 * CD : 4 * CD], uh1[:], COPY)
                st_u[q] = u_sb

            def stage_v(q):
                u_sb = st_u.pop(q)
                vbc = VBC[0]
                # agreement: tmp = u_hat * vsum, then sum over D
                # (D-major: every fold reads/writes stride-1 bf16 -> 2x)
                tmp = tp.tile([128, 4 * CD], BF16, tag="tmp")
                nc.vector.tensor_tensor(
                    out=tmp[:].rearrange("p (q n) -> p q n", n=CD),
                    in0=u_sb[:].rearrange("p (q n) -> p q n", n=CD),
                    in1=vbc[:].unsqueeze(1).broadcast_to([128, 4, CD]),
                    op=MUL,
                )
                t1 = tp.tile([128, 4 * 8 * C], BF16, tag="t1")
                v4 = tmp[:].rearrange("p (q d c) -> p q d c", d=D, c=C)
                o1 = t1[:].rearrange("p (q d c) -> p q d c", d=8, c=C)
                nc.vector.tensor_tensor(
                    out=o1, in0=v4[:, :, 0:8, :], in1=v4[:, :, 8:16, :], op=ADD
                )
                t2 = sp.tile([128, 4 * 4 * C], BF16, tag="t2")
                i2 = t1[:].rearrange("p (q d c) -> p q d c", d=8, c=C)
                o2 = t2[:].rearrange("p (q d c) -> p q d c", d=4, c=C)
                nc.vector.tensor_tensor(
                    out=o2, in0=i2[:, :, 0:4, :], in1=i2[:, :, 4:8, :], op=ADD
                )
                t3 = sp.tile([128, 4 * 2 * C], BF16, tag="t3")
                i3 = t2[:].rearrange("p (q d c) -> p q d c", d=4, c=C)
                o3 = t3[:].rearrange("p (q d c) -> p q d c", d=2, c=C)
                nc.gpsimd.tensor_tensor(
                    out=o3, in0=i3[:, :, 0:2, :], in1=i3[:, :, 2:4, :], op=ADD
                )
                a = sp.tile([128, 4 * C], F32, tag="a")
                i4 = t3[:].rearrange("p (q d c) -> p q d c", d=2, c=C)
                nc.gpsimd.tensor_tensor(
                    out=a[:].rearrange("p (q c) -> p q c", c=C).unsqueeze(2),
                    in0=i4[:, :, 0:1, :],
                    in1=i4[:, :, 1:2, :],
                    op=ADD,
                )
                st_a[q] = (u_sb, a)

            def stage_b(q):
                u_sb, a = st_a.pop(q)
                e = sp.tile([128, 4 * C], F32, tag="e")
                sm = sp.tile([128, 4], F32, tag="sm")
                cn = sp.tile([128, 4 * C], BF16, tag="cn")
                for k in range(4):
                    # exp with free-dim accumulate -> per-pair softmax denom
                    nc.scalar.activation(
                        e[:, k * C : (k + 1) * C],
                        a[:, k * C : (k + 1) * C],
                        EXP,
                        accum_out=sm[:, k : k + 1],
                    )
                rs = sp.tile([128, 4], F32, tag="rs")
                nc.vector.reciprocal(rs[:], sm[:])
                nc.vector.tensor_tensor(
                    out=cn[:].rearrange("p (q c) -> p q c", c=C),
                    in0=e[:].rearrange("p (q c) -> p q c", c=C),
                    in1=rs[:].unsqueeze(2).broadcast_to([128, 4, C]),
                    op=MUL,
                )
                st_b[q] = (u_sb, cn)

            def stage_y(q):
                u_sb, cn = st_b.pop(q)
                y = yp.tile([128, 4 * CD], BF16, tag="y")
                # cn broadcast along D stays stride-1 in the D-major layout
                nc.vector.tensor_tensor(
                    out=y[:].rearrange("p (q d c) -> p q d c", d=D, c=C),
                    in0=u_sb[:].rearrange("p (q d c) -> p q d c", d=D, c=C),
                    in1=cn[:]
                    .rearrange("p (q c) -> p q c", c=C)
                    .unsqueeze(2)
                    .broadcast_to([128, 4, D, C]),
                    op=MUL,
                )
                st_c[q] = y

            def stage_c(q):
                y = st_c.pop(q)
                sA, sB = SPS[0]
                half = NQ // 2
                s_ps = sA if q < half else sB
                for h in range(4):
                    nc.tensor.matmul(
                        s_ps[:],
                        lhsT=sel[:],
                        rhs=y[:, h * CD : (h + 1) * CD],
                        start=(q % half == 0 and h == 0),
                        stop=(q % half == half - 1 and h == 3),
                        skip_group_check=True,
                    )

            srA = half_reduce(1, s_psA, "A")
            for q in range(PRE):
                stage_u(q)
            srB = half_reduce(1, s_psB, "B")
            VBC[0] = boundary(1, srA, srB)
            HALF = NQ // 2
            for r in (2, 3):
                sA = saccp.tile([B, CD], F32, tag="spsA", name="spsA_r")
                sB = saccp.tile([B, CD], F32, tag="spsB", name="spsB_r")
                SPS[0] = (sA, sB)
                srA = None
                for qq in range(0, NQ + 8, 2):
                    for q in (qq, qq + 1):
                        if q + PRE < NQ:
                            stage_u(q + PRE)
                    for q in (qq, qq + 1):
                        if q < NQ:
                            stage_v(q)
                    for q in (qq - 2, qq - 1):
                        if 0 <= q < NQ:
                            stage_b(q)
                    for q in (qq - 4, qq - 3):
                        if 0 <= q < NQ:
                            stage_y(q)
                    for q in (qq - 6, qq - 5):
                        if 0 <= q < NQ:
                            stage_c(q)
                    if qq - 6 == HALF - 2:
                        # first-half s complete: launch its all-reduce now
                        srA = half_reduce(r, sA, "A")
                if r == 2:
                    for q in range(PRE):
                        stage_u(q)
                srB = half_reduce(r, sB, "B")
                VBC[0] = boundary(r, srA, srB)
    _split_waits(nc)
    return nc


def _prep_core_inputs(x_np, w_np, core):
    """x_np [B, I, d] f32; w_np [I, C, D, d] f32 -> per-core bf16 operands.

    Free-dim layout for u_hat tiles is D-major: n = D*C + c."""
    import ml_dtypes

    lo = core * I_LOC
    xk = x_np[:, lo : lo + I_LOC, :]  # [B, 512, 16]
    wk = w_np[lo : lo + I_LOC]  # [512, C, D, d]

    # W rows per i: [d, (D, c)] (D-major free)
    w_free = wk.transpose(0, 3, 2, 1).reshape(I_LOC, DSMALL, CD)  # [i, d, (D c)]
    # pair tiles [NPAIR, 32, CD]; rows 0:16 = i0 over d, 16:32 = i1
    wt = np.zeros((NPAIR, 32, CD), dtype=np.float32)
    wt[:, 0:DSMALL, :] = w_free[0::2]
    wt[:, DSMALL:32, :] = w_free[1::2]
    # strip-pack: pair p=4g+s -> partitions [32s,32s+32), free block g
    wsb = wt.reshape(NGRP, 4, 32, CD).transpose(1, 2, 0, 3).reshape(128, NGRP * CD)

    # x block-diag pair tiles: [NPAIR, 32, 128]
    xdg = np.zeros((NPAIR, 32, 128), dtype=np.float32)
    xT = xk.transpose(1, 2, 0)  # [i, d, B]
    xdg[:, 0:DSMALL, 0:B] = xT[0::2]
    xdg[:, DSMALL:32, B : 2 * B] = xT[1::2]
    xsb = xdg.reshape(NGRP, 4, 32, 128).transpose(1, 2, 0, 3).reshape(128, NGRP * 128)

    # dense x chunks for round 1: rows (j, d) = i 8g+j; cols (g, b)
    xs = xT.reshape(NGRP, 8 * DSMALL, B).transpose(1, 0, 2).reshape(128, NGRP * B)

    return {
        "xd": xsb.astype(ml_dtypes.bfloat16),
        "wt": wsb.astype(ml_dtypes.bfloat16),
        "xs": xs.astype(ml_dtypes.bfloat16),
    }


_NC_CACHE = {}


def kernel(x: np.ndarray, weights: np.ndarray) -> np.ndarray:
    import ml_dtypes

    x = np.asarray(x, dtype=np.float32)
    w = np.asarray(weights, dtype=np.float32)[0]  # [I, C, D, d]

    if "nc" not in _NC_CACHE:
        _NC_CACHE["nc"] = build_bass()
    nc = _NC_CACHE["nc"]

    selmask = np.zeros((128, B), dtype=np.float32)
    for p in range(128):
        selmask[p, p % B] = 1.0

    in_maps = []
    for core in range(NCORES):
        m = _prep_core_inputs(x, w, core)
        m["sel"] = selmask.astype(ml_dtypes.bfloat16)
        in_maps.append(m)

    res = run_bass_kernel_spmd(nc, in_maps, list(range(NCORES)))
    out = np.asarray(res.results[0]["out"], dtype=np.float32)  # [B, CD]
    return out.reshape(B, C, D)
